# revision 1
# baseline (speedup 1.0000x reference)
"""Deformable conv (DCNv2 pack) + sync BatchNorm + ReLU on 8 Trainium2 NeuronCores.

Strategy (data-parallel, B*rowhalf sharding -> 8 shards of 64 output rows):
  Pass 1 (per core):
    - 3x3 offset conv on PE (channel-major), transpose to pixel-major on PE
    - coordinate/bilinear-coefficient pipeline on DVE (fp32)
    - dma_gather of precomputed "quad" slots (4 bilinear corners x 64ch, fp16)
      from HBM: one 512B slot per (tap, pixel)
    - bilinear combine = per-pixel coef multiply + corner sums (DVE, fp16)
    - PE transposes to channel-major + main einsum matmuls (fp16 -> fp32 PSUM)
    - BN batch statistics via PE gram/sum matmuls
  Host: combines per-core stats into exact batch mean/var (sync BN all-reduce)
  Pass 2 (per core): y = relu(out * a[ch] + b[ch]) elementwise.

The conv bias cancels exactly in BN (shift-invariance), so it is never used.
"""
import math
import numpy as np

import concourse.bass as bass
import concourse.tile as tile
import concourse.mybir as mybir
from concourse import bacc, bass_utils, library_config
from concourse._compat import with_exitstack

F32 = mybir.dt.float32
F16 = mybir.dt.float16
I16 = mybir.dt.int16
AF = mybir.ActivationFunctionType
ALU = mybir.AluOpType

# problem geometry
B, C, O, H, W = 4, 64, 64, 128, 128
K, KH, KW = 9, 3, 3
BN_EPS = 1e-5
NCORES = 8
ROWS = 64              # output rows per core
NBLK = 8               # row-blocks per core
BROWS = ROWS // NBLK   # rows per block = 8
PIXB = BROWS * W       # pixels per block = 1024
SLOT_G = 132           # quad slot grid is SLOT_G x SLOT_G
NSLOT = SLOT_G * SLOT_G
ELEM = 4 * C           # fp16 values per slot (512B)
NIDX_B = K * PIXB      # gather indices per block = 9216
XCROWS, XCCOLS = ROWS + 2, W + 2


def build_pass1(stage="full"):
    nc = bacc.Bacc("TRN2", target_bir_lowering=False, debug=False,
                   num_devices=NCORES, dynamic_dma_scratch_size=32768)
    xq = nc.dram_tensor("xq", [NSLOT, ELEM], F16, kind="ExternalInput")
    xc = nc.dram_tensor("xc", [C, XCROWS, XCCOLS], F16, kind="ExternalInput")
    wof = nc.dram_tensor("wof", [C, K, 27], F16, kind="ExternalInput")
    boff = nc.dram_tensor("boff", [27, 1], F32, kind="ExternalInput")
    pyb = nc.dram_tensor("pyb", [128, ROWS, K], F32, kind="ExternalInput")
    pxb = nc.dram_tensor("pxb", [128, 1, K], F32, kind="ExternalInput")
    w2 = nc.dram_tensor("w2", [128, 4, O], F16, kind="ExternalInput")
    w1 = nc.dram_tensor("w1", [C, O], F16, kind="ExternalInput")
    ident = nc.dram_tensor("ident", [128, 128], F16, kind="ExternalInput")
    out_un = nc.dram_tensor("out_un", [W, ROWS, O], F32, kind="ExternalOutput")
    stats = nc.dram_tensor("stats", [O, O + 1], F32, kind="ExternalOutput")

    with tile.TileContext(nc) as tc:
        nc.gpsimd.load_library(library_config.mlp)
        with tc.tile_pool(name="const", bufs=1) as cpool, \
             tc.tile_pool(name="coord", bufs=1) as crd, \
             tc.tile_pool(name="stps", bufs=1, space="PSUM") as stps:
            # ---- constants / inputs resident in SBUF
            xc_t = cpool.tile([C, XCROWS, XCCOLS], F16)
            nc.sync.dma_start(xc_t[:], xc[:, :, :])
            wof_t = cpool.tile([C, K, 27], F16)
            nc.sync.dma_start(wof_t[:], wof[:, :, :])
            boff_t = cpool.tile([27, 1], F32)
            nc.sync.dma_start(boff_t[:], boff[:, :])
            pyb_t = cpool.tile([128, ROWS, K], F32)
            nc.sync.dma_start(pyb_t[:], pyb[:, :, :])
            pxb_t = cpool.tile([128, 1, K], F32)
            nc.sync.dma_start(pxb_t[:], pxb[:, :, :])
            w2_t = cpool.tile([128, 4, O], F16)
            nc.sync.dma_start(w2_t[:], w2[:, :, :])
            w1_t = cpool.tile([C, O], F16)
            nc.sync.dma_start(w1_t[:], w1[:, :])
            id_t = cpool.tile([128, 128], F16)
            nc.sync.dma_start(id_t[:], ident[:, :])
            ones_t = cpool.tile([128, 1], F32)
            nc.vector.memset(ones_t[:], 1.0)

            # persistent stats accumulators (PSUM)
            if stage != "phasea":
                ps_gram = stps.tile([O, O], F32)
                ps_sum = stps.tile([O, 1], F32)

            # ---- phase A: offset conv (channel-major) + transpose + coords
            off_cm = crd.tile([27, ROWS * W], F16)
            off_pm = crd.tile([128, ROWS, 27], F32)

            def conv_half(h, cvp, otp):
                for s in range(h * 8, (h + 1) * 8):   # 8 groups of 4 rows
                    pc = cvp.tile([27, 4 * W], F32, tag="pc")
                    for t in range(K):
                        ky, kx = t // 3, t % 3
                        rv = xc_t[:, 4 * s + ky: 4 * s + ky + 4, kx: kx + W]
                        nc.tensor.matmul(pc[:], wof_t[:, t, :], rv,
                                         start=(t == 0), stop=(t == K - 1))
                    # add offset-conv bias during PSUM->SBUF copy
                    nc.scalar.activation(off_cm[:, s * 4 * W: (s + 1) * 4 * W],
                                         pc[:], AF.Identity,
                                         bias=boff_t[:, 0:1])
                for q in range(h * 8, (h + 1) * 8):   # transpose 4 rows a time
                    po = otp.tile([128, 4, 28], F16, tag="po")  # 28: 4B align
                    for i in range(4):
                        jg = q * 4 + i
                        nc.tensor.transpose(po[:, i, 0:27],
                                            off_cm[:, jg * W: (jg + 1) * W],
                                            id_t[0:27, 0:27])
                    nc.scalar.activation(
                        off_pm[:, q * 4: (q + 1) * 4, :], po[:, :, 0:27],
                        AF.Copy)

            # ---- coordinate pipeline, in halves so phase B starts early
            coefs = crd.tile([128, K, ROWS, 4, 2], F16)
            srcp = crd.tile([128, NBLK, K, BROWS], I16)
            wrapped = crd.tile([128, NBLK * K * BROWS * 8], I16)
            wv = wrapped[0:16, :].rearrange("p (f g) -> p g f", g=8)
            RC = 8388608.0  # 2**23: x+RC-RC == rne(x) for 0 <= x < 2**23
            HB = ROWS // 2   # rows per half
            HBLK = NBLK // 2

            def coords_half(h):
                r0, r1 = h * HB, (h + 1) * HB
                opm = off_pm[:, r0:r1, :]
                offv = opm[:, :, 0:18].rearrange("p j (k two) -> p j two k",
                                                 two=2)
                dy, dx = offv[:, :, 0, :], offv[:, :, 1, :]
                mlog = opm[:, :, 18:27]
                shp = [128, HB, K]

                def floor_frac(pos):
                    f0 = crd.tile(shp, F32, tag="ff0")
                    nc.vector.tensor_scalar(f0[:], pos[:], RC, RC,
                                            ALU.add, ALU.subtract)
                    over = crd.tile(shp, F32, tag="fover")
                    nc.vector.tensor_tensor(over[:], f0[:], pos[:], ALU.is_gt)
                    nc.vector.tensor_tensor(f0[:], f0[:], over[:], ALU.subtract)
                    fr = crd.tile(shp, F32, tag="ffr")
                    nc.vector.tensor_tensor(fr[:], pos[:], f0[:], ALU.subtract)
                    return f0, fr

                pys = crd.tile(shp, F32, tag="pys")
                nc.vector.tensor_tensor(pys[:], dy, pyb_t[:, r0:r1, :], ALU.add)
                nc.vector.tensor_scalar(pys[:], pys[:], 0.0, float(SLOT_G - 1),
                                        ALU.max, ALU.min)
                y0, fy = floor_frac(pys)
                idxf = crd.tile(shp, F32, tag="idxf")
                nc.vector.tensor_scalar(idxf[:], y0[:], float(SLOT_G), None,
                                        ALU.mult)

                pxs = crd.tile(shp, F32, tag="pxs")
                nc.vector.tensor_tensor(pxs[:], dx,
                                        pxb_t[:].broadcast_to([128, HB, K]),
                                        ALU.add)
                nc.vector.tensor_scalar(pxs[:], pxs[:], 0.0, float(SLOT_G - 1),
                                        ALU.max, ALU.min)
                x0, fx = floor_frac(pxs)
                nc.vector.tensor_tensor(idxf[:], idxf[:], x0[:], ALU.add)
                # cast-permute to [p, block, tap, rowloc] int16
                nc.vector.tensor_copy(
                    srcp[:, h * HBLK: (h + 1) * HBLK, :, :],
                    idxf[:].rearrange("p (b j) t -> p b t j", b=HBLK))

                m = crd.tile(shp, F32, tag="m")
                nc.scalar.activation(m[:], mlog, AF.Sigmoid)
                t1 = crd.tile(shp, F32, tag="t1")
                nc.vector.tensor_tensor(t1[:], m[:], fy[:], ALU.mult)
                w11 = crd.tile(shp, F32, tag="w11")
                nc.vector.tensor_tensor(w11[:], t1[:], fx[:], ALU.mult)
                w10 = crd.tile(shp, F32, tag="w10")
                nc.vector.tensor_tensor(w10[:], t1[:], w11[:], ALU.subtract)
                t3 = crd.tile(shp, F32, tag="t3")
                nc.vector.tensor_tensor(t3[:], m[:], t1[:], ALU.subtract)
                w01 = crd.tile(shp, F32, tag="w01")
                nc.vector.tensor_tensor(w01[:], t3[:], fx[:], ALU.mult)
                w00 = crd.tile(shp, F32, tag="w00")
                nc.vector.tensor_tensor(w00[:], t3[:], w01[:], ALU.subtract)

                # coefs duplicated in pairs so the combine multiply's
                # broadcast AP reads 2 adjacent fp16 per 32-bit -> DVE 2x
                for q, wq in enumerate((w00, w01, w10, w11)):
                    nc.vector.tensor_copy(
                        coefs[:, :, r0:r1, q, :],
                        wq[:].transpose([0, 2, 1]).unsqueeze(3)
                        .broadcast_to([128, K, HB, 2]))

                # fold to wrapped-16 layout + replicate to 128 partitions
                HC = HBLK * K * BROWS           # wrapped f-cols per half
                for g in range(8):
                    eng = nc.sync if g % 2 == 0 else nc.scalar
                    eng.dma_start(
                        wv[:, g, h * HC: (h + 1) * HC],
                        srcp[16 * g: 16 * (g + 1),
                             h * HBLK: (h + 1) * HBLK, :, :])
                for r in range(1, 8):
                    eng = nc.sync if r % 2 == 0 else nc.scalar
                    eng.dma_start(
                        wrapped[16 * r: 16 * (r + 1),
                                h * HC * 8: (h + 1) * HC * 8],
                        wrapped[0:16, h * HC * 8: (h + 1) * HC * 8])

            with tc.tile_pool(name="cvp", bufs=1, space="PSUM") as cvp, \
                 tc.tile_pool(name="otp", bufs=1, space="PSUM") as otp:
                for h in (0, 1):
                    conv_half(h, cvp, otp)
                    coords_half(h)

            # ---- phase B: gather / combine / einsum per block
            if stage == "phasea":
                st_sb = crd.tile([O, O + 1], F32)
                nc.vector.memset(st_sb[:], 0.0)
                nc.sync.dma_start(stats[:, :], st_sb[:])
                nc.sync.dma_start(out_un[:, 0:1, :],
                                  wrapped[:].bitcast(F32)[:, 0:O])
            with tc.tile_pool(name="gat", bufs=12) as gat, \
                 tc.tile_pool(name="smp", bufs=2) as smp, \
                 tc.tile_pool(name="sm2", bufs=2) as sm2, \
                 tc.tile_pool(name="oblk", bufs=2) as obl, \
                 tc.tile_pool(name="stp", bufs=2, space="PSUM") as stp, \
                 tc.tile_pool(name="eip", bufs=2, space="PSUM") as eip:
                for b in range(NBLK) if stage != "phasea" else []:
                    # bilinear combine: per-tap coef multiply + horizontal
                    # corner sums on DVE; the vertical (top+bottom) sum rides
                    # the PE transpose via PSUM accumulation.
                    sp = smp.tile([128, BROWS, K, C], F16, tag="sp")
                    sp2 = smp.tile([128, BROWS, C], F16, tag="sp2")
                    if stage == "nocombine":
                        nc.vector.memset(sp[:, 0:1, 0:1, 0:1], 0.0)
                    for t in range(K):
                        # SWDGE ring copes with <=1024 gather indices per
                        # instruction; one gather per tap (1024 idx, 512B each)
                        g = gat.tile([128, BROWS, ELEM], F16, tag="g")
                        if stage in ("nogather", "nogather_ms", "nocombine"):
                            nc.vector.memset(g[:, 0:1, 0:1], 0.25)
                        else:
                            nc.gpsimd.dma_gather(
                                g[:], xq[:, :],
                                wrapped[:, b * (NIDX_B // 16) + t * (PIXB // 16):
                                        b * (NIDX_B // 16) + (t + 1) * (PIXB // 16)],
                                num_idxs=PIXB, num_idxs_reg=PIXB, elem_size=ELEM)
                        if stage == "nocombine":
                            continue
                        gvp = g[:].rearrange(
                            "p j (q c2 two) -> p j q c2 two", q=4, two=2)
                        cf = coefs[:, t, b * BROWS: (b + 1) * BROWS, :, :] \
                            .unsqueeze(3).broadcast_to([128, BROWS, 4, C // 2, 2])
                        nc.vector.tensor_tensor(gvp, gvp, cf, ALU.mult)
                        gv = g[:].rearrange("p j (q c) -> p j q c", q=4)
                        nc.vector.tensor_tensor(sp[:, :, t, :],
                                                gv[:, :, 0, :],
                                                gv[:, :, 1, :], ALU.add)
                        nc.vector.tensor_tensor(sp2[:],
                                                gv[:, :, 2, :],
                                                gv[:, :, 3, :], ALU.add)
                        nc.vector.tensor_tensor(sp[:, :, t, :],
                                                sp[:, :, t, :],
                                                sp2[:], ALU.add)

                    ob = obl.tile([128, BROWS, O], F32)
                    if stage == "noeinsum":
                        nc.vector.memset(ob[:], 0.0)
                    for jl in ([] if stage == "noeinsum" else range(BROWS)):
                        # transpose tap-pairs to channel-major; top+bottom
                        # halves accumulate in PSUM (the vertical corner sum);
                        # all 5 groups land in one PSUM bank -> one ACT copy
                        pt = stp.tile([128, 5 * 128], F16, tag="pt")
                        for tt in range(4):
                            nc.tensor.matmul(pt[:, bass.ts(tt, 128)],
                                             sp[:, jl, 2 * tt: 2 * tt + 2, :],
                                             id_t[:], is_transpose=True,
                                             start=True, stop=True,
                                             skip_group_check=True)
                        nc.tensor.matmul(pt[0:64, bass.ts(4, 128)],
                                         sp[:, jl, 8, :], id_t[:],
                                         is_transpose=True, start=True,
                                         stop=True, skip_group_check=True)
                        st = sm2.tile([128, 5 * 128], F16, tag="st")
                        nc.scalar.activation(st[:, 0:512], pt[:, 0:512], AF.Copy)
                        nc.scalar.activation(st[0:64, 512:640],
                                             pt[0:64, 512:640], AF.Copy)

                        po = eip.tile([128, O], F32)
                        for tt in range(4):
                            nc.tensor.matmul(po[:], st[:, bass.ts(tt, 128)],
                                             w2_t[:, tt, :],
                                             start=(tt == 0), stop=False)
                        nc.tensor.matmul(po[:], st[0:64, bass.ts(4, 128)],
                                         w1_t[:],
                                         start=False, stop=True)
                        nc.vector.tensor_copy(ob[:, jl, :], po[:])

                        # BN statistics: gram (diag -> sumsq) + sums
                        first = (b == 0 and jl == 0)
                        last = (b == NBLK - 1 and jl == BROWS - 1)
                        nc.tensor.matmul(ps_gram[:], ob[:, jl, :],
                                         ob[:, jl, :],
                                         start=first, stop=last,
                                         skip_group_check=True)
                        nc.tensor.matmul(ps_sum[:], ob[:, jl, :],
                                         ones_t[:],
                                         start=first, stop=last,
                                         skip_group_check=True)

                    nc.sync.dma_start(
                        out_un[:, b * BROWS: (b + 1) * BROWS, :], ob[:])

            if stage != "phasea":
                st_sb = crd.tile([O, O + 1], F32)
                nc.vector.tensor_copy(st_sb[:, 0:O], ps_gram[:])
                nc.vector.tensor_copy(st_sb[:, O: O + 1], ps_sum[:])
                nc.sync.dma_start(stats[:, :], st_sb[:])

    nc.compile()
    return nc


def build_pass2():
    nc = bacc.Bacc("TRN2", target_bir_lowering=False, debug=False,
                   num_devices=NCORES)
    un = nc.dram_tensor("un", [W, ROWS, O], F32, kind="ExternalInput")
    ab = nc.dram_tensor("ab", [128, 2 * O], F32, kind="ExternalInput")
    fin = nc.dram_tensor("fin", [W, ROWS, O], F32, kind="ExternalOutput")
    with tile.TileContext(nc) as tc:
        with tc.tile_pool(name="c", bufs=1) as cp, \
             tc.tile_pool(name="p", bufs=3) as p:
            ab_t = cp.tile([128, 2 * O], F32)
            nc.sync.dma_start(ab_t[:], ab[:, :])
            av = ab_t[:, 0:O].unsqueeze(1).broadcast_to([128, 2 * BROWS, O])
            bv = ab_t[:, O: 2 * O].unsqueeze(1).broadcast_to([128, 2 * BROWS, O])
            DR = 2 * BROWS
            for s in range(NBLK // 2):
                t = p.tile([128, DR, O], F32)
                nc.sync.dma_start(
                    t[:].rearrange("p a c -> p (a c)"),
                    un[:, s * DR: (s + 1) * DR, :].rearrange(
                        "b a c -> b (a c)"))
                nc.vector.tensor_tensor(t[:], t[:], av, ALU.mult)
                nc.vector.tensor_tensor(t[:], t[:], bv, ALU.add)
                nc.vector.tensor_scalar(t[:], t[:], 0.0, None, ALU.max)
                nc.sync.dma_start(
                    fin[:, s * DR: (s + 1) * DR, :].rearrange(
                        "b a c -> b (a c)"),
                    t[:].rearrange("p a c -> p (a c)"))
    nc.compile()
    return nc


_cache = {}


def _programs():
    if "p1" not in _cache:
        _cache["p1"] = build_pass1()
        _cache["p2"] = build_pass2()
    return _cache["p1"], _cache["p2"]


def host_prep(x, weight, w_off, b_off):
    """Build the 8 per-core input dicts for pass 1."""
    ky = (np.arange(K) // 3).astype(np.float32)
    kx = (np.arange(K) % 3).astype(np.float32)

    # quad slot tensors, one per batch image
    xqs = []
    for b in range(B):
        xhw = np.transpose(x[b], (1, 2, 0))  # [H, W, C]
        P = np.zeros((SLOT_G + 1, SLOT_G + 1, C), np.float16)
        P[2: 2 + H, 2: 2 + W] = xhw.astype(np.float16)
        xq = np.empty((SLOT_G, SLOT_G, 2, 2, C), np.float16)
        for a in range(2):
            for bb in range(2):
                xq[:, :, a, bb, :] = P[a: a + SLOT_G, bb: bb + SLOT_G]
        xqs.append(np.ascontiguousarray(xq.reshape(NSLOT, ELEM)))

    # wof[c, t, o27] = w_off[o27, c, ky, kx], t = ky*3+kx
    wof = np.ascontiguousarray(
        np.transpose(w_off, (1, 2, 3, 0)).reshape(C, K, 27)).astype(np.float16)
    boff = b_off.reshape(27, 1).astype(np.float32)
    wr = weight.reshape(O, C, K)
    w2 = np.zeros((128, 4, O), np.float16)
    for jj in range(4):
        for i in range(2):
            w2[i * C: (i + 1) * C, jj, :] = wr[:, :, 2 * jj + i].T
    w1 = np.ascontiguousarray(wr[:, :, 8].T).astype(np.float16)
    ident = np.eye(128, dtype=np.float16)
    pxb = (np.arange(128, dtype=np.float32)[:, None, None] + 1.0 +
           kx[None, None, :]) * np.ones((128, 1, K), np.float32)

    in_maps = []
    for c in range(NCORES):
        b, rh = c // 2, c % 2
        r0 = rh * ROWS
        xcn = np.zeros((C, XCROWS, XCCOLS), np.float16)
        lo, hi = max(0, r0 - 1), min(H, r0 + ROWS + 1)
        xcn[:, lo - (r0 - 1): hi - (r0 - 1), 1: 1 + W] = \
            x[:, :, lo:hi, :][b].astype(np.float16)
        jrow = np.arange(ROWS, dtype=np.float32)
        pyb = (r0 + jrow[None, :, None] + 1.0 + ky[None, None, :]) * \
            np.ones((128, ROWS, K), np.float32)
        in_maps.append({
            "xq": xqs[b], "xc": xcn, "wof": wof, "boff": boff,
            "pyb": np.ascontiguousarray(pyb),
            "pxb": np.ascontiguousarray(pxb),
            "w2": w2, "w1": w1, "ident": ident,
        })
    return in_maps


def kernel(x, weight, bias, w_off, b_off, gamma, beta):
    x = np.asarray(x, dtype=np.float32)
    weight = np.asarray(weight, dtype=np.float32)
    bias = np.asarray(bias, dtype=np.float32)
    w_off = np.asarray(w_off, dtype=np.float32)
    b_off = np.asarray(b_off, dtype=np.float32)
    gamma = np.asarray(gamma, dtype=np.float32)
    beta = np.asarray(beta, dtype=np.float32)

    p1, p2 = _programs()
    in_maps = host_prep(x, weight, w_off, b_off)
    res1 = bass_utils.run_bass_kernel_spmd(p1, in_maps, core_ids=list(range(NCORES)))

    # sync-BN all-reduce on host (exact, fp64)
    tot_sum = np.zeros(O, np.float64)
    tot_sq = np.zeros(O, np.float64)
    for c in range(NCORES):
        st = res1.results[c]["stats"].astype(np.float64)
        tot_sum += st[:, O]
        tot_sq += np.diag(st[:, 0:O])
    N = float(B * H * W)
    mean = tot_sum / N
    var = tot_sq / N - mean ** 2
    a = (gamma.astype(np.float64) / np.sqrt(var + BN_EPS))
    bsh = beta.astype(np.float64) - mean * a
    ab = np.zeros((128, 2 * O), np.float32)
    ab[:, 0:O] = a.astype(np.float32)
    ab[:, O:] = bsh.astype(np.float32)

    in_maps2 = [{"un": res1.results[c]["out_un"], "ab": ab}
                for c in range(NCORES)]
    res2 = bass_utils.run_bass_kernel_spmd(p2, in_maps2, core_ids=list(range(NCORES)))

    out = np.empty((B, O, H, W), np.float32)
    for c in range(NCORES):
        b, rh = c // 2, c % 2
        fin = res2.results[c]["fin"]  # [W, ROWS, O]
        out[b, :, rh * ROWS: (rh + 1) * ROWS, :] = np.transpose(fin, (2, 1, 0))
    return out



# revision 18
# speedup vs baseline: 111.2253x; 111.2253x over previous
"""Deformable conv (DCNv2 pack) + sync BatchNorm + ReLU on 8 Trainium2 NeuronCores.

Strategy (data-parallel, B*rowhalf sharding -> 8 shards of 64 output rows):
  Pass 1 (per core):
    - 3x3 offset conv on PE (channel-major), transpose to pixel-major on PE
    - coordinate/bilinear-coefficient pipeline on DVE (fp32)
    - dma_gather of precomputed "quad" slots (4 bilinear corners x 64ch, fp16)
      from HBM: one 512B slot per (tap, pixel)
    - bilinear combine = per-pixel coef multiply + corner sums (DVE, fp16)
    - PE transposes to channel-major + main einsum matmuls (fp16 -> fp32 PSUM)
    - BN batch statistics via PE gram/sum matmuls
  Host: combines per-core stats into exact batch mean/var (sync BN all-reduce)
  Pass 2 (per core): y = relu(out * a[ch] + b[ch]) elementwise.

The conv bias cancels exactly in BN (shift-invariance), so it is never used.
"""
import math
import numpy as np

import concourse.bass as bass
import concourse.tile as tile
import concourse.mybir as mybir
from concourse import bacc, bass_utils, library_config
from concourse._compat import with_exitstack

F32 = mybir.dt.float32
F16 = mybir.dt.float16
I16 = mybir.dt.int16
AF = mybir.ActivationFunctionType
ALU = mybir.AluOpType

# problem geometry
B, C, O, H, W = 4, 64, 64, 128, 128
K, KH, KW = 9, 3, 3
BN_EPS = 1e-5
NCORES = 8
ROWS = 64              # output rows per core
NBLK = 8               # row-blocks per core
BROWS = ROWS // NBLK   # rows per block = 8
PIXB = BROWS * W       # pixels per block = 1024
SLOT_G = 132           # quad slot grid is SLOT_G x SLOT_G
NSLOT = SLOT_G * SLOT_G
ELEM = 4 * C           # fp16 values per slot (512B)
NIDX_B = K * PIXB      # gather indices per block = 9216
XCROWS, XCCOLS = ROWS + 2, W + 2


def build_pass1(stage="full"):
    nc = bacc.Bacc("TRN2", target_bir_lowering=False, debug=False,
                   num_devices=NCORES, dynamic_dma_scratch_size=32768)
    xq = nc.dram_tensor("xq", [NSLOT, ELEM], F16, kind="ExternalInput")
    xc = nc.dram_tensor("xc", [C, XCROWS, XCCOLS], F16, kind="ExternalInput")
    wof = nc.dram_tensor("wof", [C, K, 27], F16, kind="ExternalInput")
    boff = nc.dram_tensor("boff", [27, 1], F32, kind="ExternalInput")
    pyb = nc.dram_tensor("pyb", [128, ROWS, K], F32, kind="ExternalInput")
    pxb = nc.dram_tensor("pxb", [128, 1, K], F32, kind="ExternalInput")
    w2 = nc.dram_tensor("w2", [128, 4, O], F16, kind="ExternalInput")
    w1 = nc.dram_tensor("w1", [C, O], F16, kind="ExternalInput")
    ident = nc.dram_tensor("ident", [128, 128], F16, kind="ExternalInput")
    out_un = nc.dram_tensor("out_un", [W, ROWS, O], F16, kind="ExternalOutput")
    stats = nc.dram_tensor("stats", [O, O + 1], F32, kind="ExternalOutput")

    with tile.TileContext(nc) as tc:
        nc.gpsimd.load_library(library_config.mlp)
        with tc.tile_pool(name="const", bufs=1) as cpool, \
             tc.tile_pool(name="coord", bufs=1) as crd, \
             tc.tile_pool(name="stps", bufs=1, space="PSUM") as stps:
            # ---- constants / inputs resident in SBUF
            xc_t = cpool.tile([C, XCROWS, XCCOLS], F16)
            nc.sync.dma_start(xc_t[:], xc[:, :, :])
            wof_t = cpool.tile([C, K, 27], F16)
            nc.sync.dma_start(wof_t[:], wof[:, :, :])
            boff_t = cpool.tile([27, 1], F32)
            nc.sync.dma_start(boff_t[:], boff[:, :])
            pyb_t = cpool.tile([128, ROWS, K], F32)
            nc.sync.dma_start(pyb_t[:], pyb[:, :, :])
            pxb_t = cpool.tile([128, 1, K], F32)
            nc.sync.dma_start(pxb_t[:], pxb[:, :, :])
            w2_t = cpool.tile([128, 4, O], F16)
            nc.sync.dma_start(w2_t[:], w2[:, :, :])
            w1_t = cpool.tile([C, O], F16)
            nc.sync.dma_start(w1_t[:], w1[:, :])
            id_t = cpool.tile([128, 128], F16)
            nc.sync.dma_start(id_t[:], ident[:, :])
            ones_t = cpool.tile([128, 1], F16)
            nc.vector.memset(ones_t[:], 1.0)

            # persistent stats accumulators (PSUM)
            if stage != "phasea":
                ps_gram = stps.tile([O, O], F32)
                ps_sum = stps.tile([O, 1], F32)

            # ---- phase A: offset conv (channel-major) + transpose + coords
            off_cm = crd.tile([27, ROWS * W], F16)
            off_pm = crd.tile([128, ROWS, 27], F32)

            def conv_half(h, cvp, otp):
                for s in range(h * 8, (h + 1) * 8):   # 8 groups of 4 rows
                    pc = cvp.tile([27, 4 * W], F32, tag="pc")
                    for t in range(K):
                        ky, kx = t // 3, t % 3
                        rv = xc_t[:, 4 * s + ky: 4 * s + ky + 4, kx: kx + W]
                        nc.tensor.matmul(pc[:], wof_t[:, t, :], rv,
                                         start=(t == 0), stop=(t == K - 1))
                    # add offset-conv bias during PSUM->SBUF copy
                    nc.scalar.activation(off_cm[:, s * 4 * W: (s + 1) * 4 * W],
                                         pc[:], AF.Identity,
                                         bias=boff_t[:, 0:1])
                for q in range(h * 8, (h + 1) * 8):   # transpose 4 rows a time
                    po = otp.tile([128, 4, 28], F16, tag="po")  # 28: 4B align
                    for i in range(4):
                        jg = q * 4 + i
                        nc.tensor.transpose(po[:, i, 0:27],
                                            off_cm[:, jg * W: (jg + 1) * W],
                                            id_t[0:27, 0:27])
                    nc.scalar.activation(
                        off_pm[:, q * 4: (q + 1) * 4, :], po[:, :, 0:27],
                        AF.Copy)

            # ---- coordinate pipeline, in halves so phase B starts early
            # coefs2 block-major so one DVE mult covers a whole block
            coefs = crd.tile([128, NBLK, K, BROWS, 4, 2], F16)
            srcp = crd.tile([128, NBLK, K, BROWS], I16)
            wrapped = crd.tile([128, NBLK * K * BROWS * 8], I16)
            wv = wrapped[0:16, :].rearrange("p (f g) -> p g f", g=8)
            RC = 8388608.0  # 2**23: x+RC-RC == rne(x) for 0 <= x < 2**23
            HB = ROWS // 2   # rows per half
            HBLK = NBLK // 2

            def coords_half(h):
                r0, r1 = h * HB, (h + 1) * HB
                opm = off_pm[:, r0:r1, :]
                offv = opm[:, :, 0:18].rearrange("p j (k two) -> p j two k",
                                                 two=2)
                dy, dx = offv[:, :, 0, :], offv[:, :, 1, :]
                mlog = opm[:, :, 18:27]
                shp = [128, HB, K]

                def floor_frac(pos):
                    f0 = crd.tile(shp, F32, tag="ff0")
                    nc.vector.tensor_scalar(f0[:], pos[:], RC, RC,
                                            ALU.add, ALU.subtract)
                    over = crd.tile(shp, F32, tag="fover")
                    nc.vector.tensor_tensor(over[:], f0[:], pos[:], ALU.is_gt)
                    nc.vector.tensor_tensor(f0[:], f0[:], over[:], ALU.subtract)
                    fr = crd.tile(shp, F32, tag="ffr")
                    nc.vector.tensor_tensor(fr[:], pos[:], f0[:], ALU.subtract)
                    return f0, fr

                pys = crd.tile(shp, F32, tag="pys")
                nc.vector.tensor_tensor(pys[:], dy, pyb_t[:, r0:r1, :], ALU.add)
                nc.vector.tensor_scalar(pys[:], pys[:], 0.0, float(SLOT_G - 1),
                                        ALU.max, ALU.min)
                y0, fy = floor_frac(pys)
                idxf = crd.tile(shp, F32, tag="idxf")
                nc.vector.tensor_scalar(idxf[:], y0[:], float(SLOT_G), None,
                                        ALU.mult)

                pxs = crd.tile(shp, F32, tag="pxs")
                nc.vector.tensor_tensor(pxs[:], dx,
                                        pxb_t[:].broadcast_to([128, HB, K]),
                                        ALU.add)
                nc.vector.tensor_scalar(pxs[:], pxs[:], 0.0, float(SLOT_G - 1),
                                        ALU.max, ALU.min)
                x0, fx = floor_frac(pxs)
                nc.vector.tensor_tensor(idxf[:], idxf[:], x0[:], ALU.add)
                # cast-permute to [p, block, tap, rowloc] int16
                nc.vector.tensor_copy(
                    srcp[:, h * HBLK: (h + 1) * HBLK, :, :],
                    idxf[:].rearrange("p (b j) t -> p b t j", b=HBLK))

                m = crd.tile(shp, F32, tag="m")
                nc.scalar.activation(m[:], mlog, AF.Sigmoid)
                t1 = crd.tile(shp, F32, tag="t1")
                nc.vector.tensor_tensor(t1[:], m[:], fy[:], ALU.mult)
                w11 = crd.tile(shp, F32, tag="w11")
                nc.vector.tensor_tensor(w11[:], t1[:], fx[:], ALU.mult)
                w10 = crd.tile(shp, F32, tag="w10")
                nc.vector.tensor_tensor(w10[:], t1[:], w11[:], ALU.subtract)
                t3 = crd.tile(shp, F32, tag="t3")
                nc.vector.tensor_tensor(t3[:], m[:], t1[:], ALU.subtract)
                w01 = crd.tile(shp, F32, tag="w01")
                nc.vector.tensor_tensor(w01[:], t3[:], fx[:], ALU.mult)
                w00 = crd.tile(shp, F32, tag="w00")
                nc.vector.tensor_tensor(w00[:], t3[:], w01[:], ALU.subtract)

                # coefs duplicated in pairs so the combine multiply's
                # broadcast AP reads 2 adjacent fp16 per 32-bit -> DVE 2x
                for q, wq in enumerate((w00, w01, w10, w11)):
                    nc.vector.tensor_copy(
                        coefs[:, h * HBLK: (h + 1) * HBLK, :, :, q, :],
                        wq[:].rearrange("p (b j) t -> p b t j", b=HBLK)
                        .unsqueeze(4).broadcast_to([128, HBLK, K, BROWS, 2]))

                # fold to wrapped-16 layout + replicate to 128 partitions
                HC = HBLK * K * BROWS           # wrapped f-cols per half
                for g in range(8):
                    eng = nc.sync if g % 2 == 0 else nc.scalar
                    eng.dma_start(
                        wv[:, g, h * HC: (h + 1) * HC],
                        srcp[16 * g: 16 * (g + 1),
                             h * HBLK: (h + 1) * HBLK, :, :])
                for r in range(1, 8):
                    eng = nc.sync if r % 2 == 0 else nc.scalar
                    eng.dma_start(
                        wrapped[16 * r: 16 * (r + 1),
                                h * HC * 8: (h + 1) * HC * 8],
                        wrapped[0:16, h * HC * 8: (h + 1) * HC * 8])

            with tc.tile_pool(name="cvp", bufs=1, space="PSUM") as cvp, \
                 tc.tile_pool(name="otp", bufs=1, space="PSUM") as otp:
                for h in (0, 1):
                    conv_half(h, cvp, otp)
                    coords_half(h)

            # ---- phase B: gather / combine / einsum per block
            if stage == "phasea":
                st_sb = crd.tile([O, O + 1], F32)
                nc.vector.memset(st_sb[:], 0.0)
                nc.sync.dma_start(stats[:, :], st_sb[:])
                nc.sync.dma_start(out_un[:, 0:1, :],
                                  wrapped[:].bitcast(F16)[:, 0:O])
            with tc.tile_pool(name="gat", bufs=2) as gat, \
                 tc.tile_pool(name="smp", bufs=2) as smp, \
                 tc.tile_pool(name="sm2", bufs=2) as sm2, \
                 tc.tile_pool(name="oblk", bufs=1) as obl, \
                 tc.tile_pool(name="stp", bufs=2, space="PSUM") as stp, \
                 tc.tile_pool(name="eip", bufs=2, space="PSUM") as eip:
                # whole-output staging tile, fp16, resident across blocks
                if stage != "phasea":
                    obf = obl.tile([128, NBLK, BROWS, O], F16)
                for b in range(NBLK) if stage != "phasea" else []:
                    # one big gather tile per block: 9 taps land in tap-slices,
                    # then the bilinear combine is 4 wide DVE ops (1 coef
                    # multiply + 3 corner adds) instead of 36 narrow ones.
                    G = gat.tile([128, K, BROWS, ELEM], F16, tag="G")
                    sp = smp.tile([128, BROWS, K, C], F16, tag="sp")
                    if stage in ("nogather", "nocombine"):
                        nc.vector.memset(G[:, 0:1, 0:1, 0:1], 0.25)
                    else:
                        for t in range(K):
                            # SWDGE ring copes with <=1024 gather indices per
                            # instruction; one gather per tap (1024 idx x 512B)
                            nc.gpsimd.dma_gather(
                                G[:, t], xq[:, :],
                                wrapped[:, b * (NIDX_B // 16) + t * (PIXB // 16):
                                        b * (NIDX_B // 16) + (t + 1) * (PIXB // 16)],
                                num_idxs=PIXB, num_idxs_reg=PIXB, elem_size=ELEM)
                    if stage == "nocombine":
                        nc.vector.memset(sp[:, 0:1, 0:1, 0:1], 0.0)
                    else:
                        gvp = G[:].rearrange(
                            "p t j (q c2 two) -> p (t j) q c2 two", q=4, two=2)
                        cf = coefs[:, b].rearrange(
                            "p t j q two -> p (t j) q two").unsqueeze(3) \
                            .broadcast_to([128, K * BROWS, 4, C // 2, 2])
                        nc.vector.tensor_tensor(gvp, gvp, cf, ALU.mult)
                        gv = G[:].rearrange("p t j (q c) -> p j t q c", q=4)
                        nc.vector.tensor_tensor(sp[:], gv[:, :, :, 0, :],
                                                gv[:, :, :, 1, :], ALU.add)
                        # corner 2+3 sums in place into G's corner-3 slice
                        nc.vector.tensor_tensor(gv[:, :, :, 3, :],
                                                gv[:, :, :, 2, :],
                                                gv[:, :, :, 3, :], ALU.add)
                        nc.vector.tensor_tensor(sp[:], sp[:],
                                                gv[:, :, :, 3, :], ALU.add)

                    ob = obf[:, b]
                    if stage == "noeinsum":
                        nc.vector.memset(ob[:, 0:1, 0:1], 0.0)
                    for jl in ([] if stage == "noeinsum" else range(BROWS)):
                        # transpose tap-pairs to channel-major; top+bottom
                        # halves accumulate in PSUM (the vertical corner sum);
                        # all 5 groups land in one PSUM bank -> one ACT copy
                        pt = stp.tile([128, 5 * 128], F16, tag="pt")
                        for tt in range(4):
                            nc.tensor.matmul(pt[:, bass.ts(tt, 128)],
                                             sp[:, jl, 2 * tt: 2 * tt + 2, :],
                                             id_t[:], is_transpose=True,
                                             start=True, stop=True,
                                             skip_group_check=True)
                        nc.tensor.matmul(pt[0:64, bass.ts(4, 128)],
                                         sp[:, jl, 8, :], id_t[:],
                                         is_transpose=True, start=True,
                                         stop=True, skip_group_check=True)
                        st = sm2.tile([128, 5 * 128], F16, tag="st")
                        nc.scalar.activation(st[:, 0:512], pt[:, 0:512], AF.Copy)
                        nc.scalar.activation(st[0:64, 512:640],
                                             pt[0:64, 512:640], AF.Copy)

                        po = eip.tile([128, O], F32)
                        for tt in range(4):
                            nc.tensor.matmul(po[:], st[:, bass.ts(tt, 128)],
                                             w2_t[:, tt, :],
                                             start=(tt == 0), stop=False)
                        nc.tensor.matmul(po[:], st[0:64, bass.ts(4, 128)],
                                         w1_t[:],
                                         start=False, stop=True)
                        # PSUM drain on ACT (f32 -> f16) keeps DVE free
                        nc.scalar.activation(ob[:, jl, :], po[:], AF.Copy)

                        # BN statistics: gram (diag -> sumsq) + sums
                        first = (b == 0 and jl == 0)
                        last = (b == NBLK - 1 and jl == BROWS - 1)
                        nc.tensor.matmul(ps_gram[:], ob[:, jl, :],
                                         ob[:, jl, :],
                                         start=first, stop=last,
                                         skip_group_check=True)
                        nc.tensor.matmul(ps_sum[:], ob[:, jl, :],
                                         ones_t[:],
                                         start=first, stop=last,
                                         skip_group_check=True)

                    nc.sync.dma_start(
                        out_un[:, b * BROWS: (b + 1) * BROWS, :], ob[:])

            if stage == "noeinsum":
                st_sb = crd.tile([O, O + 1], F32)
                nc.vector.memset(st_sb[:], 0.0)
                nc.sync.dma_start(stats[:, :], st_sb[:])
            elif stage != "phasea":
                st_sb = crd.tile([O, O + 1], F32)
                nc.vector.tensor_copy(st_sb[:, 0:O], ps_gram[:])
                nc.vector.tensor_copy(st_sb[:, O: O + 1], ps_sum[:])
                nc.sync.dma_start(stats[:, :], st_sb[:])

    nc.compile()
    return nc


def build_pass2():
    nc = bacc.Bacc("TRN2", target_bir_lowering=False, debug=False,
                   num_devices=NCORES)
    un = nc.dram_tensor("un", [W, ROWS, O], F16, kind="ExternalInput")
    ab = nc.dram_tensor("ab", [128, 2 * O], F16, kind="ExternalInput")
    fin = nc.dram_tensor("fin", [W, ROWS, O], F16, kind="ExternalOutput")
    with tile.TileContext(nc) as tc:
        with tc.tile_pool(name="c", bufs=1) as cp, \
             tc.tile_pool(name="p", bufs=3) as p:
            ab_t = cp.tile([128, 2 * O], F16)
            nc.sync.dma_start(ab_t[:], ab[:, :])
            av = ab_t[:, 0:O].unsqueeze(1).broadcast_to([128, 2 * BROWS, O])
            bv = ab_t[:, O: 2 * O].unsqueeze(1).broadcast_to([128, 2 * BROWS, O])
            DR = 2 * BROWS
            for s in range(NBLK // 2):
                t = p.tile([128, DR, O], F16)
                nc.sync.dma_start(
                    t[:].rearrange("p a c -> p (a c)"),
                    un[:, s * DR: (s + 1) * DR, :].rearrange(
                        "b a c -> b (a c)"))
                nc.vector.tensor_tensor(t[:], t[:], av, ALU.mult)
                nc.vector.tensor_tensor(t[:], t[:], bv, ALU.add)
                nc.vector.tensor_scalar(t[:], t[:], 0.0, None, ALU.max)
                nc.sync.dma_start(
                    fin[:, s * DR: (s + 1) * DR, :].rearrange(
                        "b a c -> b (a c)"),
                    t[:].rearrange("p a c -> p (a c)"))
    nc.compile()
    return nc


_cache = {}


def _programs():
    if "p1" not in _cache:
        _cache["p1"] = build_pass1()
        _cache["p2"] = build_pass2()
    return _cache["p1"], _cache["p2"]


def host_prep(x, weight, w_off, b_off):
    """Build the 8 per-core input dicts for pass 1."""
    ky = (np.arange(K) // 3).astype(np.float32)
    kx = (np.arange(K) % 3).astype(np.float32)

    # quad slot tensors, one per batch image
    xqs = []
    for b in range(B):
        xhw = np.transpose(x[b], (1, 2, 0))  # [H, W, C]
        P = np.zeros((SLOT_G + 1, SLOT_G + 1, C), np.float16)
        P[2: 2 + H, 2: 2 + W] = xhw.astype(np.float16)
        xq = np.empty((SLOT_G, SLOT_G, 2, 2, C), np.float16)
        for a in range(2):
            for bb in range(2):
                xq[:, :, a, bb, :] = P[a: a + SLOT_G, bb: bb + SLOT_G]
        xqs.append(np.ascontiguousarray(xq.reshape(NSLOT, ELEM)))

    # wof[c, t, o27] = w_off[o27, c, ky, kx], t = ky*3+kx
    wof = np.ascontiguousarray(
        np.transpose(w_off, (1, 2, 3, 0)).reshape(C, K, 27)).astype(np.float16)
    boff = b_off.reshape(27, 1).astype(np.float32)
    wr = weight.reshape(O, C, K)
    w2 = np.zeros((128, 4, O), np.float16)
    for jj in range(4):
        for i in range(2):
            w2[i * C: (i + 1) * C, jj, :] = wr[:, :, 2 * jj + i].T
    w1 = np.ascontiguousarray(wr[:, :, 8].T).astype(np.float16)
    ident = np.eye(128, dtype=np.float16)
    pxb = (np.arange(128, dtype=np.float32)[:, None, None] + 1.0 +
           kx[None, None, :]) * np.ones((128, 1, K), np.float32)

    in_maps = []
    for c in range(NCORES):
        b, rh = c // 2, c % 2
        r0 = rh * ROWS
        xcn = np.zeros((C, XCROWS, XCCOLS), np.float16)
        lo, hi = max(0, r0 - 1), min(H, r0 + ROWS + 1)
        xcn[:, lo - (r0 - 1): hi - (r0 - 1), 1: 1 + W] = \
            x[:, :, lo:hi, :][b].astype(np.float16)
        jrow = np.arange(ROWS, dtype=np.float32)
        pyb = (r0 + jrow[None, :, None] + 1.0 + ky[None, None, :]) * \
            np.ones((128, ROWS, K), np.float32)
        in_maps.append({
            "xq": xqs[b], "xc": xcn, "wof": wof, "boff": boff,
            "pyb": np.ascontiguousarray(pyb),
            "pxb": np.ascontiguousarray(pxb),
            "w2": w2, "w1": w1, "ident": ident,
        })
    return in_maps


def prep_pass2_maps(in_maps):
    """Timing-only helper: dummy input maps for pass 2."""
    ab = np.zeros((128, 2 * O), np.float16)
    un = np.zeros((W, ROWS, O), np.float16)
    return [{"un": un, "ab": ab} for _ in range(NCORES)]


def kernel(x, weight, bias, w_off, b_off, gamma, beta):
    x = np.asarray(x, dtype=np.float32)
    weight = np.asarray(weight, dtype=np.float32)
    bias = np.asarray(bias, dtype=np.float32)
    w_off = np.asarray(w_off, dtype=np.float32)
    b_off = np.asarray(b_off, dtype=np.float32)
    gamma = np.asarray(gamma, dtype=np.float32)
    beta = np.asarray(beta, dtype=np.float32)

    p1, p2 = _programs()
    in_maps = host_prep(x, weight, w_off, b_off)
    res1 = bass_utils.run_bass_kernel_spmd(p1, in_maps, core_ids=list(range(NCORES)))

    # sync-BN all-reduce on host (exact, fp64)
    tot_sum = np.zeros(O, np.float64)
    tot_sq = np.zeros(O, np.float64)
    for c in range(NCORES):
        st = res1.results[c]["stats"].astype(np.float64)
        tot_sum += st[:, O]
        tot_sq += np.diag(st[:, 0:O])
    N = float(B * H * W)
    mean = tot_sum / N
    var = tot_sq / N - mean ** 2
    a = (gamma.astype(np.float64) / np.sqrt(var + BN_EPS))
    bsh = beta.astype(np.float64) - mean * a
    ab = np.zeros((128, 2 * O), np.float16)
    ab[:, 0:O] = a.astype(np.float16)
    ab[:, O:] = bsh.astype(np.float16)

    in_maps2 = [{"un": res1.results[c]["out_un"], "ab": ab}
                for c in range(NCORES)]
    res2 = bass_utils.run_bass_kernel_spmd(p2, in_maps2, core_ids=list(range(NCORES)))

    out = np.empty((B, O, H, W), np.float32)
    for c in range(NCORES):
        b, rh = c // 2, c % 2
        fin = res2.results[c]["fin"].astype(np.float32)  # [W, ROWS, O]
        out[b, :, rh * ROWS: (rh + 1) * ROWS, :] = np.transpose(fin, (2, 1, 0))
    return out

